# revision 18
# baseline (speedup 1.0000x reference)
"""Trainium2 Bass kernel for nn_DecoderBlock (B=1, S=4096, D=1024, H=16, dh=64).

Strategy (8 NeuronCores, SPMD, no collectives):
  Launch 1 — attention, tensor-parallel over heads (2 heads/core).
    Host precomputes hT = rmsnorm(x)^T in fp8 (e4m3) and 32x-scaled per-core
    head weight slices in fp8. On-chip:
      phase A: Q/K/V projections as fp8 DoubleRow matmuls (2 d-tiles
        contracted per instruction), casts Q,K to bf16 (scale 1/32) and V to
        fp8 via a bf16 PE transpose into "paired" AV layout [128k, 2, 65]
        (65th column = ones for the softmax denominator).
      phase B: causal streaming softmax. Scores = bf16 matmuls (contract=64).
        exp is split: even tiles on ScalarE (true exp -> fp8, scale 1/8,
        bias -3), odd tiles on VectorE via a Schraudolph fast-exp writing
        e4m3 *bits* as saturating uint8 (clamps negatives to +0). The -3
        logit shift keeps exp() < 240 (TRN e4m3 max); it cancels in the
        host-side num/den divide. Diagonal-tile causal masking is a zero-fill
        affine_select on the int8 view. AV = fp8 DoubleRow matmuls (contract
        = 2 k-tiles x 128) accumulating unnormalized o and denominators.
  Host — divides by denominators, assembles o^T.
  Launch 2 — sequence-parallel (512 tokens/core): o@WO in fp8 DoubleRow
    (o,WO scaled 64x/32x; un-scaled in the residual STT add), residual +
    rmsnorm in f32, MLP in bf16 (fp8 would breach the 2e-2 gate), residual.

Precision: attention is ~1% of the output magnitude (residual dilution), so
fp8 there is harmless; the MLP dominates and stays bf16.
"""

import sys
import types

import numpy as np
import ml_dtypes


# ---------------------------------------------------------------------------
# Environment compatibility shims (inlined — kernel.py must be self-contained)
# ---------------------------------------------------------------------------
def _install_compat():
    try:
        import trn_agent_boot.trn_boot as _tb

        if "antenv.axon_hooks" not in sys.modules:
            _hook = _tb._ntff_profile_via_ctypes("/opt/axon/libaxon_pjrt.so")
            _m = types.ModuleType("antenv.axon_hooks")
            _m.get_axon_ntff_profile_hook = lambda: _hook
            sys.modules["antenv.axon_hooks"] = _m
    except Exception:
        pass

    import concourse.mybir as mybir
    from concourse import tile as _tile
    from concourse import bass_utils as _bass_utils
    from concourse.vector_clock import ScopedClock as _ScopedClock

    _bass_utils.upload_artifacts = lambda tmpdir: f"local:{tmpdir}"

    def _patched_drain_and_barrier(self, tick_clock, wait_clock):
        nc = self.nc
        drain_inst = nc.sync.drain()
        wait_clock.add_sem_waits(
            drain_inst.ins, _ScopedClock({None: tick_clock.global_clock})
        )
        si = drain_inst.ins.sync_info
        waits = list(si.on_wait or []) if si else []
        if len(waits) > 1:
            drain_inst.ins.sync_info = mybir.SyncInfo(
                on_wait=waits[:1], on_update=list(si.on_update or [])
            )
            for i in range(1, len(waits)):
                nop = nc.sync.nop(nofuse=True, hint="drain_wait_split")
                nop.ins.sync_info = mybir.SyncInfo(on_wait=waits[i : i + 1], on_update=[])
        nc.all_engine_barrier()
        assert self.sems is not None
        popped = nc._tile_sem_poison_stack.pop()
        assert popped is self._sem_poison
        nc.clear_and_free_semaphores(list(self.sems.allocated().values()))
        nc.all_engine_barrier()

    _tile.TileContext._drain_and_barrier = _patched_drain_and_barrier


_install_compat()

import concourse.bass as bass
import concourse.mybir as mybir
from concourse import tile
from concourse.masks import make_identity
from concourse.bass_utils import run_bass_kernel_spmd

F32 = mybir.dt.float32
F32R = mybir.dt.float32r
BF16 = mybir.dt.bfloat16
FP8 = mybir.dt.float8e4
I8 = mybir.dt.int8
U8 = mybir.dt.uint8
AF = mybir.ActivationFunctionType
ALU = mybir.AluOpType
DR = mybir.MatmulPerfMode.DoubleRow

S, D, H, DH = 4096, 1024, 16, 64
NCORES = 8
HPC = H // NCORES          # heads per core = 2
SC = S // NCORES           # seq chunk per core = 512
NCH = S // SC              # number of 512-chunks = 8
NDT = D // 128             # d-tiles = 8
NDP = NDT // 2             # d-tile pairs (DoubleRow) = 4
EPS = 1e-6
WSCALE = 32.0              # fp8 weight pre-scale (values ~N(0,0.02) -> ~0.64)
OSCALE = 64.0              # fp8 o pre-scale for launch 2
ESHIFT = 3.0               # logit shift: p = exp(logit - 3); cancels in num/den
# Schraudolph fast-exp on raw scores s (logit = s/8):
#   e4m3 bits = clamp_u8(round(s * FE_A + FE_C));  tuned numerically.
FE_A = 1.44269504
FE_C = 20.915319


def _split_multi_waits(nc, max_waits: int = 1):
    """This walrus build accepts only one sem-wait per instruction; hoist
    extras onto fresh NoOps inserted just before, on the same engine."""
    n_split = 0
    for fn in nc.m.functions:
        for blk in fn.blocks:
            out = []
            changed = False
            for inst in blk.instructions:
                si = inst.sync_info
                waits = list(si.on_wait or []) if si else []
                if len(waits) > max_waits:
                    changed = True
                    for i in range(0, len(waits) - max_waits, max_waits):
                        nop = mybir.InstNoOp(
                            name=f"I-waitsplit-{n_split}", ins=[], outs=[]
                        )
                        n_split += 1
                        nop.engine = inst.engine
                        nop.sync_info = mybir.SyncInfo(
                            on_wait=waits[i : i + max_waits], on_update=[]
                        )
                        out.append(nop)
                    inst.sync_info = mybir.SyncInfo(
                        on_wait=waits[len(waits) - max_waits :],
                        on_update=list(si.on_update or []),
                    )
                out.append(inst)
            if changed:
                blk.instructions = out
    return n_split


# ---------------------------------------------------------------------------
# Launch 1: head-sharded attention (fp8 projections/AV, bf16 scores)
# ---------------------------------------------------------------------------
def build_l1():
    nc = bass.Bass("TRN2", target_bir_lowering=False, debug=False)
    ht8 = nc.declare_dram_parameter("ht8", [NCH, 128, NDT, SC], FP8, isOutput=False)
    wq8 = nc.declare_dram_parameter("wq8", [D, 128], FP8, isOutput=False)
    wk8 = nc.declare_dram_parameter("wk8", [D, 128], FP8, isOutput=False)
    wv8 = nc.declare_dram_parameter("wv8", [D, 128], FP8, isOutput=False)
    oden = nc.declare_dram_parameter("oden", [HPC, DH + 1, S], F32, isOutput=True)

    with tile.TileContext(nc) as tc:
        with (
            tc.tile_pool(name="const", bufs=1) as const,
            tc.tile_pool(name="wsb", bufs=1) as wsb,
            tc.tile_pool(name="big", bufs=1) as big,
            tc.tile_pool(name="hsb", bufs=3) as hsb,
            tc.tile_pool(name="vt", bufs=3) as vt_pool,
            tc.tile_pool(name="pp", bufs=4) as pp,
            tc.tile_pool(name="stg", bufs=2) as stg_pool,
        ):
            idf = const.tile([128, 64], F32)
            make_identity(nc, idf[0:64, :])
            make_identity(nc, idf[64:128, :])
            ident = const.tile([128, 64], BF16)
            nc.vector.tensor_copy(ident[:], idf[:])
            negsh = const.tile([128, 1], F32)
            nc.vector.memset(negsh[:], -ESHIFT)

            wq_sb = wsb.tile([128, NDT, 128], FP8)
            wk_sb = wsb.tile([128, NDT, 128], FP8)
            wv_sb = wsb.tile([128, NDT, 128], FP8)
            for wp, wt in ((wq8, wq_sb), (wk8, wk_sb), (wv8, wv_sb)):
                nc.sync.dma_start(
                    out=wt[:], in_=wp[:].rearrange("(a p) f -> p a f", p=128)
                )

            # DR fp8 layouts [128, 2, S]: j=0 holds data, j=1 is zeros.
            # kt8_h holds head h's K on its own 64 rows; other rows zero, so a
            # full-contract DR matmul with the 2-head qt8 picks out head h.
            qt8 = big.tile([128, 2, S], FP8)
            kt8_0 = big.tile([128, 2, S], FP8)
            kt8_1 = big.tile([128, 2, S], FP8)
            zsrc = big.tile([128, 1024], FP8)
            nc.vector.memset(zsrc[:], 0.0)
            # AV stationary: [128 k, chunk, head, pair, j, 128]; col 64 = ones,
            # cols 65..127 = zeros (DoubleRow needs 64/128-wide tiles).
            vc_sb = big.tile([128, NCH, HPC, 2, 2, 128], FP8)
            nc.vector.memset(vc_sb[:, :, :, :, :, DH : DH + 1], 1.0)

            # ---- phase A: projections; V via DMA-transpose + gpsimd fp8 cast
            with tc.tile_pool(name="psA", bufs=1, space="PSUM") as psA:
                for qc in range(NCH):
                    h_c = hsb.tile([128, NDT, SC], FP8, tag="hsb")
                    nc.sync.dma_start(out=h_c[:], in_=ht8[qc])
                    sl = slice(qc * SC, (qc + 1) * SC)
                    nc.scalar.dma_start(out=qt8[:, 1, sl], in_=zsrc[:, 0:SC])
                    nc.scalar.dma_start(out=kt8_0[:, 1, sl], in_=zsrc[:, 0:SC])
                    nc.scalar.dma_start(out=kt8_1[:, 1, sl], in_=zsrc[:, 0:SC])
                    nc.scalar.dma_start(out=kt8_0[64:128, 0, sl],
                                        in_=zsrc[64:128, 0:SC])
                    nc.scalar.dma_start(out=kt8_1[0:64, 0, sl],
                                        in_=zsrc[0:64, 0:SC])
                    nc.scalar.dma_start(
                        out=vc_sb[:, qc, :, :, :, DH + 1 : 128],
                        in_=zsrc[:, 0:504].rearrange(
                            "p (a b c d) -> p a b c d", a=2, b=2, c=2
                        ),
                    )
                    prjs = []
                    for wt in (wq_sb, wk_sb, wv_sb):
                        ps = psA.tile([128, SC], F32, tag="psp", bufs=6,
                                      name=f"prj_{qc}_{len(prjs)}")
                        for dp in range(NDP):
                            nc.tensor.matmul(
                                ps[:],
                                wt[:, 2 * dp : 2 * dp + 2, :],
                                h_c[:, 2 * dp : 2 * dp + 2, :],
                                start=(dp == 0),
                                stop=(dp == NDP - 1),
                                perf_mode=DR,
                            )
                        prjs.append(ps)
                    nc.vector.tensor_scalar_mul(
                        qt8[:, 0, sl], prjs[0][:], 1.0 / WSCALE
                    )
                    nc.vector.tensor_scalar_mul(
                        kt8_0[0:64, 0, sl], prjs[1][0:64, :], 1.0 / WSCALE
                    )
                    nc.vector.tensor_scalar_mul(
                        kt8_1[64:128, 0, sl], prjs[1][64:128, :], 1.0 / WSCALE
                    )
                    vt_c = vt_pool.tile([128, SC], BF16, tag="vt")
                    nc.scalar.activation(
                        vt_c[:], prjs[2][:], AF.Copy, scale=1.0 / WSCALE
                    )
                    for hh in range(HPC):
                        vcb = vt_pool.tile([128, 4, DH], BF16, tag="vcb",
                                           bufs=4, name=f"vcb_{qc}_{hh}")
                        eng = nc.sync if hh == 0 else nc.scalar
                        eng.dma_start_transpose(
                            out=vcb[:], in_=vt_c[hh * 64 : (hh + 1) * 64, :]
                        )
                        nc.gpsimd.tensor_copy(
                            vc_sb[:, qc, hh, :, :, 0:DH], vcb[:]
                        )

            # ---- phase B: causal attention over k-tile pairs
            with tc.tile_pool(name="psB", bufs=1, space="PSUM") as psB:
                pairctr = 0
                for qc in range(NCH):
                    npr = 2 * (qc + 1)
                    o_ps = [
                        psB.tile([128, SC], F32, tag=f"o{hh}", bufs=1,
                                 name=f"o_ps_{qc}_{hh}")
                        for hh in range(HPC)
                    ]
                    for pr in range(npr):
                        s_tiles = []
                        for hh in range(HPC):
                            s_ps = psB.tile([128, 2, SC], F32, tag="sps", bufs=3,
                                            name=f"s_ps_{qc}_{pr}_{hh}")
                            kt8_h = kt8_0 if hh == 0 else kt8_1
                            for j in range(2):
                                kt = 2 * pr + j
                                nc.tensor.matmul(
                                    s_ps[:, j, :],
                                    kt8_h[:, :, kt * 128 : (kt + 1) * 128],
                                    qt8[:, :, qc * SC : (qc + 1) * SC],
                                    start=True,
                                    stop=True,
                                    perf_mode=DR,
                                )
                            s_tiles.append(s_ps)
                        for hh in range(HPC):
                            p_t = pp.tile([128, 2, SC], FP8, tag="pp",
                                          name=f"p_{qc}_{pr}_{hh}")
                            if (pairctr + hh) % 2 == 0:
                                nc.scalar.activation(
                                    p_t[:], s_tiles[hh][:], AF.Exp,
                                    scale=0.125, bias=negsh[:],
                                )
                            else:
                                nc.vector.tensor_scalar(
                                    out=p_t[:].bitcast(U8),
                                    in0=s_tiles[hh][:],
                                    scalar1=FE_A, scalar2=FE_C,
                                    op0=ALU.mult, op1=ALU.add,
                                )
                            for j in range(2):
                                kt = 2 * pr + j
                                if kt >= 4 * qc:  # diagonal tile
                                    c0 = 128 * kt - 512 * qc
                                    if c0 > 0:  # cols [0, c0): k_min > q -> all zero
                                        nc.gpsimd.memset(
                                            p_t[:, j, 0:c0].bitcast(I8), 0
                                        )
                                    # boundary subtile [c0, c0+128): triangular
                                    nc.gpsimd.affine_select(
                                        out=p_t[:, j, c0 : c0 + 128].bitcast(I8),
                                        in_=p_t[:, j, c0 : c0 + 128].bitcast(I8),
                                        compare_op=ALU.is_ge,
                                        fill=0.0,
                                        base=0,
                                        pattern=[[1, 128]],
                                        channel_multiplier=-1,
                                    )
                            nc.tensor.matmul(
                                o_ps[hh][:],
                                vc_sb[:, pr // 2, hh, pr % 2, :, :],
                                p_t[:],
                                start=(pr == 0),
                                stop=(pr == npr - 1),
                                perf_mode=DR,
                            )
                        pairctr += 1
                    for hh in range(HPC):
                        stg = stg_pool.tile([DH + 1, SC], F32, tag="stg")
                        if hh == 0:
                            nc.scalar.copy(stg[:], o_ps[hh][0 : DH + 1, :])
                        else:
                            nc.vector.tensor_copy(stg[:], o_ps[hh][0 : DH + 1, :])
                        nc.sync.dma_start(
                            out=oden[hh, :, qc * SC : (qc + 1) * SC], in_=stg[:]
                        )

    _split_multi_waits(nc)
    return nc


# ---------------------------------------------------------------------------
# Launch 2: sequence-sharded  WO(fp8) + residual + rmsnorm + MLP(bf16)
# ---------------------------------------------------------------------------
def build_l2():
    nc = bass.Bass("TRN2", target_bir_lowering=False, debug=False)
    xt = nc.declare_dram_parameter("xt", [128, NDT, SC], F32, isOutput=False)
    ot8 = nc.declare_dram_parameter("ot8", [128, NDT, SC], FP8, isOutput=False)
    wo8 = nc.declare_dram_parameter("wo8", [NDT, 128, NDT, 128], FP8, isOutput=False)
    w1 = nc.declare_dram_parameter("w1", [32, 128, NDT, 128], BF16, isOutput=False)
    w2 = nc.declare_dram_parameter("w2", [NDT, 4, 128, NDT, 128], BF16, isOutput=False)
    b1 = nc.declare_dram_parameter("b1", [128, 32], F32, isOutput=False)
    b2 = nc.declare_dram_parameter("b2", [128, 8], F32, isOutput=False)
    yt = nc.declare_dram_parameter("yt", [D, SC], F32, isOutput=True)

    NHT = 4 * D // 128  # 32 hidden tiles

    with tile.TileContext(nc) as tc:
        with (
            tc.tile_pool(name="const", bufs=1) as const,
            tc.tile_pool(name="big", bufs=1) as big,
            tc.tile_pool(name="wt", bufs=6) as wt_pool,
            tc.tile_pool(name="a1", bufs=NHT) as a1_pool,
            tc.tile_pool(name="sq", bufs=2) as sq_pool,
            tc.tile_pool(name="y", bufs=2) as y_pool,
            tc.tile_pool(name="psa", bufs=4, space="PSUM") as ps_a,
            tc.tile_pool(name="psn", bufs=2, space="PSUM") as ps_n,
        ):
            ones_f = const.tile([128, 1], F32)
            nc.vector.memset(ones_f[:], 1.0)
            ones_r = const.tile([128, 1], F32R)
            nc.vector.tensor_copy(ones_r[:], ones_f[:])
            ones_row_f = const.tile([1, 128], F32)
            nc.vector.memset(ones_row_f[:], 1.0)
            ones_row = const.tile([1, 128], F32R)
            nc.vector.tensor_copy(ones_row[:], ones_row_f[:])
            eps_t = const.tile([1, 1], F32)
            nc.vector.memset(eps_t[:], EPS)
            b1_sb = const.tile([128, 32], F32)
            nc.sync.dma_start(out=b1_sb[:], in_=b1[:])
            b2_sb = const.tile([128, 8], F32)
            nc.sync.dma_start(out=b2_sb[:], in_=b2[:])

            xt_sb = big.tile([128, NDT, SC], F32)
            ot_sb = big.tile([128, NDT, SC], FP8)
            xm_sb = big.tile([128, NDT, SC], F32)
            h2_sb = big.tile([128, NDT, SC], BF16)
            for dt in range(NDT):
                eng = nc.sync if dt % 2 == 0 else nc.scalar
                eng.dma_start(out=ot_sb[:, dt, :], in_=ot8[:, dt, :])
            for dt in range(NDT):
                eng = nc.sync if dt % 2 == 1 else nc.scalar
                eng.dma_start(out=xt_sb[:, dt, :], in_=xt[:, dt, :])

            def slab_dma(idx, src_ap, dtype, tag):
                w_t = wt_pool.tile([128, NDT, 128], dtype, tag=tag,
                                   name=f"wslab_{idx}")
                eng = nc.sync if idx % 2 == 0 else nc.scalar
                eng.dma_start(out=w_t[:], in_=src_ap)
                return w_t

            # x_mid^T = x^T + (WO^T @ o^T) / (OSCALE*WSCALE)   [fp8 DoubleRow]
            for do in range(NDT):
                w_t = slab_dma(do, wo8[do], FP8, "wo8s")
                ps = ps_a.tile([128, SC], F32, tag="psa", name=f"wops_{do}")
                for dp in range(NDP):
                    nc.tensor.matmul(
                        ps[:],
                        w_t[:, 2 * dp : 2 * dp + 2, :],
                        ot_sb[:, 2 * dp : 2 * dp + 2, :],
                        start=(dp == 0),
                        stop=(dp == NDP - 1),
                        perf_mode=DR,
                    )
                nc.vector.scalar_tensor_tensor(
                    out=xm_sb[:, do, :], in0=ps[:],
                    scalar=1.0 / (OSCALE * WSCALE), in1=xt_sb[:, do, :],
                    op0=ALU.mult, op1=ALU.add,
                )
                # rmsnorm accumulation interleaved with the WO stage
                sq = sq_pool.tile([128, SC], F32R, tag="sq")
                nc.vector.tensor_mul(sq[:], xm_sb[:, do, :], xm_sb[:, do, :])
                if do == 0:
                    ps_sum = ps_n.tile([1, SC], F32, tag="psn")
                nc.tensor.matmul(
                    ps_sum[:], ones_r[:], sq[:], start=(do == 0),
                    stop=(do == NDT - 1),
                )

            rt = sq_pool.tile([1, SC], F32, tag="rt")
            nc.scalar.activation(
                rt[:], ps_sum[:], AF.Sqrt, bias=eps_t[:], scale=1.0 / D
            )
            rr = sq_pool.tile([1, SC], F32R, tag="rt")
            with nc.allow_low_precision(reason="f32r rounding of rmsnorm scale"):
                nc.vector.reciprocal(rr[:], rt[:])
            ps_b = ps_n.tile([128, SC], F32, tag="psn")
            nc.tensor.matmul(ps_b[:], ones_row[:], rr[:], start=True, stop=True)
            for dt in range(NDT):
                nc.vector.tensor_mul(
                    h2_sb[:, dt, :], xm_sb[:, dt, :], ps_b[:]
                )

            # MLP up + relu (bf16)
            a1_tiles = []
            for ht_i in range(NHT):
                w_t = slab_dma(NDT + ht_i, w1[ht_i], BF16, "wslab")
                ps = ps_a.tile([128, SC], F32, tag="psa", name=f"w1ps_{ht_i}")
                for dt in range(NDT):
                    nc.tensor.matmul(
                        ps[:],
                        w_t[:, dt, :],
                        h2_sb[:, dt, :],
                        start=(dt == 0),
                        stop=(dt == NDT - 1),
                    )
                a1 = a1_pool.tile([128, SC], BF16, tag="a1")
                a1_tiles.append(a1)
                nc.scalar.activation(
                    a1[:], ps[:], AF.Relu, bias=b1_sb[:, ht_i : ht_i + 1], scale=1.0
                )

            # MLP down + bias + residual (bf16)
            for do in range(NDT):
                ps = ps_a.tile([128, SC], F32, tag="psa", name=f"w2ps_{do}")
                for s4 in range(4):
                    w_t = slab_dma(NDT + NHT + do * 4 + s4, w2[do, s4],
                                   BF16, "wslab")
                    for a in range(NDT):
                        ht_i = s4 * NDT + a
                        nc.tensor.matmul(
                            ps[:],
                            w_t[:, a, :],
                            a1_tiles[ht_i][:],
                            start=(ht_i == 0),
                            stop=(ht_i == NHT - 1),
                        )
                y = y_pool.tile([128, SC], F32, tag="y")
                nc.vector.scalar_tensor_tensor(
                    out=y[:], in0=ps[:], scalar=1.0, in1=xm_sb[:, do, :],
                    op0=ALU.mult, op1=ALU.add,
                )
                nc.vector.tensor_scalar_add(y[:], y[:], b2_sb[:, do : do + 1])
                nc.sync.dma_start(out=yt[do * 128 : (do + 1) * 128, :], in_=y[:])

    _split_multi_waits(nc)
    return nc


_NC_L1 = None
_NC_L2 = None


def _get_programs():
    global _NC_L1, _NC_L2
    if _NC_L1 is None:
        _NC_L1 = build_l1()
        _NC_L2 = build_l2()
    return _NC_L1, _NC_L2


def _f8(x):
    return np.asarray(np.clip(x, -240.0, 240.0), dtype=ml_dtypes.float8_e4m3)


def _bf(x):
    return np.asarray(x, dtype=ml_dtypes.bfloat16)


def _prep_l1(x, g1, WQ, WK, WV):
    ms = (x * x).mean(-1)
    r1 = 1.0 / np.sqrt(ms + EPS)
    hT = (x * r1[:, None]).T
    # [NCH, 128, NDT, SC]: per-chunk, per-partition-contiguous slabs
    ht8 = _f8(np.ascontiguousarray(
        hT.reshape(NDT, 128, NCH, SC).transpose(2, 1, 0, 3)))
    fq = WSCALE * (g1[None, :, None] * WQ)
    fk = WSCALE * (g1[None, :, None] * WK)
    fv = WSCALE * (g1[None, :, None] * WV)
    in_maps = []
    for i in range(NCORES):
        in_maps.append(
            {
                "ht8": ht8,
                "wq8": _f8(np.concatenate([fq[2 * i], fq[2 * i + 1]], axis=1)),
                "wk8": _f8(np.concatenate([fk[2 * i], fk[2 * i + 1]], axis=1)),
                "wv8": _f8(np.concatenate([fv[2 * i], fv[2 * i + 1]], axis=1)),
            }
        )
    return in_maps


def _prep_l2(x, oden_results, g2, WO, W1, B1, W2, B2):
    oT = np.empty((D, S), np.float32)
    for i in range(NCORES):
        od = oden_results[i]["oden"]
        for hh in range(HPC):
            g = 2 * i + hh
            oT[g * DH : (g + 1) * DH] = od[hh, :DH] / od[hh, DH : DH + 1]
    xT = x.T
    wo8 = _f8(np.ascontiguousarray(
        (WSCALE * WO).reshape(NDT, 128, NDT, 128).transpose(2, 1, 0, 3)))
    w1b = _bf(np.ascontiguousarray(
        (g2[:, None] * W1).reshape(NDT, 128, 32, 128).transpose(2, 1, 0, 3)))
    w2b = _bf(np.ascontiguousarray(
        W2.reshape(4, NDT, 128, NDT, 128).transpose(3, 0, 2, 1, 4)))
    b1r = np.ascontiguousarray(B1.reshape(32, 128).T)
    b2r = np.ascontiguousarray(B2.reshape(8, 128).T)
    in_maps = []
    for i in range(NCORES):
        sl = slice(i * SC, (i + 1) * SC)
        in_maps.append(
            {
                "xt": np.ascontiguousarray(
                    xT[:, sl].reshape(NDT, 128, SC).transpose(1, 0, 2)),
                "ot8": _f8(OSCALE * oT[:, sl].reshape(NDT, 128, SC)
                           .transpose(1, 0, 2)),
                "wo8": wo8,
                "w1": w1b,
                "w2": w2b,
                "b1": b1r,
                "b2": b2r,
            }
        )
    return in_maps


def kernel(**inputs):
    x = np.asarray(inputs["x"], dtype=np.float32)[0]
    g1 = np.asarray(inputs["g1"], dtype=np.float32)
    g2 = np.asarray(inputs["g2"], dtype=np.float32)
    WQ = np.asarray(inputs["WQ"], dtype=np.float32)
    WK = np.asarray(inputs["WK"], dtype=np.float32)
    WV = np.asarray(inputs["WV"], dtype=np.float32)
    WO = np.asarray(inputs["WO"], dtype=np.float32)
    W1 = np.asarray(inputs["W1"], dtype=np.float32)
    B1 = np.asarray(inputs["B1"], dtype=np.float32)
    W2 = np.asarray(inputs["W2"], dtype=np.float32)
    B2 = np.asarray(inputs["B2"], dtype=np.float32)

    nc1, nc2 = _get_programs()
    core_ids = list(range(NCORES))

    in1 = _prep_l1(x, g1, WQ, WK, WV)
    res1 = run_bass_kernel_spmd(nc1, in1, core_ids).results

    in2 = _prep_l2(x, res1, g2, WO, W1, B1, W2, B2)
    res2 = run_bass_kernel_spmd(nc2, in2, core_ids).results

    yT = np.concatenate([res2[i]["yt"] for i in range(NCORES)], axis=1)
    return np.ascontiguousarray(yT.T).reshape(1, S, D)
